# revision 26
# baseline (speedup 1.0000x reference)
"""GRU cell on 8 Trainium2 NeuronCores.

Reference computation (B=65536, D=256):
    z = sigmoid(x@Wz + h@Uz + bz)
    r = sigmoid(x@Wr + h@Ur + br)
    h_hat = tanh(x@Wh + (r*h)@Uh + bh)
    h_t = z*h + (1-z)*h_hat  ; returns (h_t, h_t)

Strategy: data-parallel over the batch dim (8 shards of 8192 rows).
The host pre-transposes and pre-blocks each shard so every on-chip
tensor lives in [hidden, batch] layout (contraction dim on the SBUF
partition dim, no on-chip transposes, biases as per-partition ACT bias
vectors).  All six GEMMs run in float32r (full-rate PE mode for fp32
data at moving-dim >= 256); only the elementwise blend
h_t = hh + z*(h - hh) runs in bf16 (measured 2.5e-3 rel-l2 on the full
problem, vs the 2e-2 gate), which doubles DVE throughput for it and
halves the output-store bytes.

Key optimizations vs the 125.2us starting point (all trace-measured):
  * piece-major host layouts ([16,128,2,512] per input, per-partition
    contiguous) so every load/store is one DMA instruction with 4KB
    descriptors; loads ride the sync HWDGE ring, weights+biases ride
    the scalar HWDGE ring phase-balanced so each gate's weights land
    just before its first matmul needs them.
  * output stores go through gpsimd (SWDGE) so they never queue behind
    loads; the final mini-chunk stores use the (by then idle) HWDGE
    rings for fast completion.
  * ~70 dependency-free bf16 warmup matmuls flip the PE HAM clock-gate
    to 2.4GHz during the DMA prologue and bridge until the first real
    matmul (otherwise the first ~3.4us of matmuls run at 1.2GHz, and a
    >3.4us PE idle gap re-throttles).
  * the last 2 pieces run as four 256-col mini-chunks, phase-
    interleaved (all r gates, then all z, then all candidates) so the
    rh muls reach the DVE queue before any combine work: the final
    ACT+DVE+store chain is the kernel's exit critical path.
"""

import os
import sys

for _p in ("/opt/trn_rl_repo", "/root/.axon_site/_ro/trn_rl_repo"):
    if os.path.isdir(_p) and _p not in sys.path:
        sys.path.append(_p)

import numpy as np

B = 65536
D = 256
N_CORES = 8
S = B // N_CORES  # batch rows per core
CH = 512  # batch columns per compute chunk / DMA piece
NCH = S // CH  # 16 pieces

# order of the 12 [128,256] weight slabs in the packed weight tensor
_WORDER = (("Wr", 0), ("Wr", 1), ("Ur", 0), ("Ur", 1),
           ("Wz", 0), ("Wz", 1), ("Uz", 0), ("Uz", 1),
           ("Wh", 0), ("Wh", 1), ("Uh", 0), ("Uh", 1))
_BORDER = ("br", "bz", "bh")  # bias col = 2*gate_idx + g


def build_nc(s=S, warm_mms=None):
    """Build + compile the per-core Bass program for a shard of s rows."""
    import concourse.bass as bass
    import concourse.mybir as mybir
    import concourse.tile as tile
    from concourse import bacc

    f32 = mybir.dt.float32
    f32r = mybir.dt.float32r
    bf16 = mybir.dt.bfloat16
    AF = mybir.ActivationFunctionType
    if warm_mms is None:
        warm_mms = int(os.environ.get("GRU_WARM", "70"))

    nch = s // CH
    nc = bacc.Bacc("TRN2", target_bir_lowering=False,
                   enable_partition_id=False)
    xP = nc.dram_tensor("xP", [nch, 128, 2, CH], f32, kind="ExternalInput")
    hP = nc.dram_tensor("hP", [nch, 128, 2, CH], f32, kind="ExternalInput")
    wp_d = nc.dram_tensor("wpack", [128, 12, 256], f32, kind="ExternalInput")
    bp_d = nc.dram_tensor("bpack", [128, 6], f32, kind="ExternalInput")
    oP = nc.dram_tensor("outP", [nch, 128, 2, CH], bf16, kind="ExternalOutput")

    xPr = xP.bitcast(f32r)
    hPr = hP.bitcast(f32r)
    wp_r = wp_d.bitcast(f32r)

    with tile.TileContext(nc) as tc:
        with (
            tc.tile_pool(name="const", bufs=1) as cpool,
            tc.tile_pool(name="inp", bufs=8) as ipool,
            tc.tile_pool(name="work", bufs=3) as wpool,
            tc.tile_pool(name="psum", bufs=1, space=bass.MemorySpace.PSUM) as ppool,
        ):
            # --- PE warmup: dependency-free bf16 matmuls flip the HAM
            # clock gate to 2.4GHz while the DMA prologue runs.
            pwarm = ppool.tile([128, 64], f32, tag="pwarm")
            if warm_mms:
                warm = cpool.tile([128, 128], bf16, tag="warm")
                nc.vector.memset(warm[:], 0.0)
                for _ in range(warm_mms):
                    nc.tensor.matmul(pwarm[:], warm[:, 0:128],
                                     warm[:, 0:64], start=True, stop=True)

            wp_sb = cpool.tile([128, 12, 256], f32r, tag="wpack")
            b_sb = cpool.tile([128, 6], f32, tag="bpack")

            def load_piece(c):
                xc = ipool.tile([128, 2, CH], f32r, tag="x")
                nc.sync.dma_start(xc[:], xPr[c])
                hc = ipool.tile([128, 2, CH], f32r, tag="h")
                nc.sync.dma_start(hc[:], hPr[c])
                return xc, hc

            # prologue, phase-balanced across the two HWDGE rings so each
            # gate's weights land just before its first matmul needs them:
            #   phase1 (gates first MM): sync x0+h0 | scalar Wr,Ur + bias
            #   phase2 (z gate):        sync Wz    | scalar Uz
            #   phase3 (candidate):     sync Wh    | scalar Uh
            #   phase4+:                sync x1,h1,x2,...
            nc.scalar.dma_start(wp_sb[:, 0:4, :], wp_r[:, 0:4, :])  # Wr,Ur
            nc.scalar.dma_start(b_sb[:], bp_d[:])
            xh0 = load_piece(0)
            nc.sync.dma_start(wp_sb[:, 4:6, :], wp_r[:, 4:6, :])  # Wz
            nc.scalar.dma_start(wp_sb[:, 6:8, :], wp_r[:, 6:8, :])  # Uz
            nc.sync.dma_start(wp_sb[:, 10:12, :], wp_r[:, 10:12, :])  # Uh
            nc.scalar.dma_start(wp_sb[:, 8:10, :], wp_r[:, 8:10, :])  # Wh
            x1 = ipool.tile([128, 2, CH], f32r, tag="x", name="x1")
            nc.sync.dma_start(x1[:], xPr[1])
            h1 = ipool.tile([128, 2, CH], f32r, tag="h", name="h1")
            nc.scalar.dma_start(h1[:], hPr[1])
            xh1 = (x1, h1)

            def wsl(j, g):
                """Stationary [128,128] slab: weight j, output half g."""
                return wp_sb[:, j, g * 128:(g + 1) * 128]

            def gate_psum(p, psl, jw, ju, rhs_w, rhs_u, g):
                """p[:, psl] = W[:,g].T @ rhs_w + U[:,g].T @ rhs_u."""
                nc.tensor.matmul(p[:, psl], wsl(jw, g), rhs_w(0),
                                 start=True, stop=False)
                nc.tensor.matmul(p[:, psl], wsl(jw + 1, g), rhs_w(1),
                                 start=False, stop=False)
                nc.tensor.matmul(p[:, psl], wsl(ju, g), rhs_u(0),
                                 start=False, stop=False)
                nc.tensor.matmul(p[:, psl], wsl(ju + 1, g), rhs_u(1),
                                 start=False, stop=True)

            def palloc(tag):
                return ppool.tile([128, CH], f32, tag=tag, name=tag)

            def mk(piece, lo, w):
                xc, hc = piece
                csl = slice(lo, lo + w)
                return (lambda k: xc[:, k, csl],
                        lambda k: hc[:, k, csl],
                        lambda k: hc[:, k, csl].bitcast(f32))

            def r_gate(xk, hk, hf, psl, g):
                pr = palloc(f"pr{g}")
                gate_psum(pr, psl, 0, 2, xk, hk, g)
                rt = wpool.tile([128, CH], f32, tag=f"r{g}", name="rt")
                nc.scalar.activation(rt[:, psl], pr[:, psl], AF.Sigmoid,
                                     bias=b_sb[:, g:g + 1])
                t = wpool.tile([128, CH], f32r, tag=f"rh{g}", name="rhr")
                nc.vector.tensor_mul(t[:, psl], rt[:, psl], hf(g))
                return t

            def z_gate(xk, hk, psl, g):
                pz = palloc(f"pz{g}")
                gate_psum(pz, psl, 4, 6, xk, hk, g)
                t = wpool.tile([128, CH], f32, tag=f"z{g}", name="zt")
                nc.scalar.activation(t[:, psl], pz[:, psl], AF.Sigmoid,
                                     bias=b_sb[:, 2 + g:3 + g])
                return t

            def h_gate(xk, rh, hf, zt, o, psl, g):
                """candidate + bf16 blend into o[:, g, psl]"""
                ph = palloc(f"ph{g}")
                gate_psum(ph, psl, 8, 10, xk, lambda k: rh[k][:, psl], g)
                hh = wpool.tile([128, CH], f32, tag=f"hh{g}", name="hh")
                nc.scalar.activation(hh[:, psl], ph[:, psl], AF.Tanh,
                                     bias=b_sb[:, 4 + g:5 + g])
                dt_ = wpool.tile([128, CH], f32, tag=f"d{g}", name="dt")
                nc.vector.tensor_sub(dt_[:, psl], hf(g), hh[:, psl])
                mt = wpool.tile([128, CH], f32, tag=f"m{g}", name="mt")
                nc.vector.tensor_mul(mt[:, psl], zt[g][:, psl], dt_[:, psl])
                nc.vector.tensor_add(o[:, g, psl], hh[:, psl], mt[:, psl])

            def compute(piece, c, lo, w, store_eng):
                """One compute chunk: batch cols [lo, lo+w) of piece c."""
                xk, hk, hf = mk(piece, lo, w)
                psl = slice(0, w)
                rh = [r_gate(xk, hk, hf, psl, g) for g in range(2)]
                zt = [z_gate(xk, hk, psl, g) for g in range(2)]
                o = wpool.tile([128, 2, CH], bf16, tag="o", name="ot")
                for g in range(2):
                    h_gate(xk, rh, hf, zt, o, psl, g)
                store_eng.dma_start(oP[c, :, :, lo:lo + w], o[:, :, psl])

            def compute_tail(items):
                """Tail items (piece, c, lo, w, store_eng) phase-interleaved:
                all r gates first, then all z, then all candidates, so the
                rh muls reach the DVE queue before any combine work and the
                PE never drains waiting on ACT/DVE products (fp32r keeps
                full PE rate down to moving-dim 256)."""
                rh, zt = {}, {}
                for i, (piece, c, lo, w, eng) in enumerate(items):
                    xk, hk, hf = mk(piece, lo, w)
                    psl = slice(0, w)
                    for g in range(2):
                        rh[i, g] = r_gate(xk, hk, hf, psl, g)
                for i, (piece, c, lo, w, eng) in enumerate(items):
                    xk, hk, hf = mk(piece, lo, w)
                    psl = slice(0, w)
                    zt[i] = [z_gate(xk, hk, psl, g) for g in range(2)]
                for i, (piece, c, lo, w, eng) in enumerate(items):
                    xk, hk, hf = mk(piece, lo, w)
                    psl = slice(0, w)
                    o = wpool.tile([128, 2, CH], bf16, tag="o", name="ot")
                    for g in range(2):
                        h_gate(xk, [rh[i, 0], rh[i, 1]], hf, zt[i], o, psl, g)
                    eng.dma_start(oP[c, :, :, lo:lo + w], o[:, :, psl])

            tail = []
            for c in range(nch):
                if c == 0:
                    piece = xh0
                elif c == 1:
                    piece = xh1
                else:
                    piece = load_piece(c)

                if c < nch - 2:
                    compute(piece, c, 0, CH, nc.gpsimd)
                elif c == nch - 2:
                    tail.append((piece, c, 0, CH // 2, nc.gpsimd))
                    tail.append((piece, c, CH // 2, CH // 2, nc.gpsimd))
                else:
                    tail.append((piece, c, 0, CH // 2, nc.sync))
                    tail.append((piece, c, CH // 2, CH // 2, nc.scalar))
            compute_tail(tail)

    nc.compile()
    return nc


_NC_CACHE = {}


def _get_nc():
    key = (S, os.environ.get("GRU_WARM", "70"))
    if key not in _NC_CACHE:
        _NC_CACHE[key] = build_nc(S)
    return _NC_CACHE[key]


def _make_in_maps(inputs):
    f32 = np.float32
    x = np.asarray(inputs["x"], f32)
    h = np.asarray(inputs["h_t_1"], f32)
    wpack = np.empty((128, 12, 256), f32)
    for j, (name, k) in enumerate(_WORDER):
        wpack[:, j, :] = np.asarray(inputs[name], f32)[k * 128:(k + 1) * 128, :]
    bpack = np.empty((128, 6), f32)
    for i, name in enumerate(_BORDER):
        b = np.asarray(inputs[name], f32)
        for g in range(2):
            bpack[:, 2 * i + g] = b[g * 128:(g + 1) * 128]
    consts = {"wpack": np.ascontiguousarray(wpack),
              "bpack": np.ascontiguousarray(bpack)}

    def pieces(a_shard):
        # [s, 256] -> [nch, 128, 2, CH]:  P[c, p, k, m] = a[c*CH+m, k*128+p]
        v = a_shard.T.reshape(2, 128, NCH, CH)  # [k, p, c, m]
        return np.ascontiguousarray(v.transpose(2, 1, 0, 3))

    in_maps = []
    for c in range(N_CORES):
        sl = slice(c * S, (c + 1) * S)
        m = {"xP": pieces(x[sl]), "hP": pieces(h[sl])}
        m.update(consts)
        in_maps.append(m)
    return in_maps


def run(inputs, trace=False):
    """Run on hardware; returns (h_t ndarray, BassKernelResults)."""
    from concourse.bass_utils import run_bass_kernel_spmd

    nc = _get_nc()
    in_maps = _make_in_maps(inputs)
    res = run_bass_kernel_spmd(nc, in_maps, list(range(N_CORES)), trace=trace)
    out = np.empty((B, D), np.float32)
    for c in range(N_CORES):
        oP = np.asarray(res.results[c]["outP"]).astype(np.float32)
        # out[c*S + cc*CH + m, g*128 + p] = oP[cc, p, g, m]
        out[c * S:(c + 1) * S] = oP.transpose(0, 3, 2, 1).reshape(S, D)
    return out, res


def kernel(**inputs):
    out, _ = run(inputs, trace=False)
    return (out, out)


# revision 27
# speedup vs baseline: 1.0322x; 1.0322x over previous
"""GRU cell on 8 Trainium2 NeuronCores.

Reference computation (B=65536, D=256):
    z = sigmoid(x@Wz + h@Uz + bz)
    r = sigmoid(x@Wr + h@Ur + br)
    h_hat = tanh(x@Wh + (r*h)@Uh + bh)
    h_t = z*h + (1-z)*h_hat  ; returns (h_t, h_t)

Strategy: data-parallel over the batch dim (8 shards of 8192 rows).
The host pre-transposes and pre-blocks each shard so every on-chip
tensor lives in [hidden, batch] layout (contraction dim on the SBUF
partition dim, no on-chip transposes, biases as per-partition ACT bias
vectors).  All six GEMMs run in float32r (full-rate PE mode for fp32
data at moving-dim >= 256); only the elementwise blend
h_t = hh + z*(h - hh) runs in bf16 (measured 2.5e-3 rel-l2 on the full
problem, vs the 2e-2 gate), which doubles DVE throughput for it and
halves the output-store bytes.

Key optimizations vs the 125.2us starting point (all trace-measured):
  * piece-major host layouts ([16,128,2,512] per input, per-partition
    contiguous) so every load/store is one DMA instruction with 4KB
    descriptors; loads ride the sync HWDGE ring, weights+biases ride
    the scalar HWDGE ring phase-balanced so each gate's weights land
    just before its first matmul needs them.
  * output stores go through gpsimd (SWDGE) so they never queue behind
    loads; the final mini-chunk stores use the (by then idle) HWDGE
    rings for fast completion.
  * ~70 dependency-free bf16 warmup matmuls flip the PE HAM clock-gate
    to 2.4GHz during the DMA prologue and bridge until the first real
    matmul (otherwise the first ~3.4us of matmuls run at 1.2GHz, and a
    >3.4us PE idle gap re-throttles).
  * the last 2 pieces run as four 256-col mini-chunks, phase-
    interleaved (all r gates, then all z, then all candidates) so the
    rh muls reach the DVE queue before any combine work: the final
    ACT+DVE+store chain is the kernel's exit critical path.
"""

import os
import sys

for _p in ("/opt/trn_rl_repo", "/root/.axon_site/_ro/trn_rl_repo"):
    if os.path.isdir(_p) and _p not in sys.path:
        sys.path.append(_p)

import numpy as np

B = 65536
D = 256
N_CORES = 8
S = B // N_CORES  # batch rows per core
CH = 512  # batch columns per compute chunk / DMA piece
NCH = S // CH  # 16 pieces

# order of the 12 [128,256] weight slabs in the packed weight tensor
_WORDER = (("Wr", 0), ("Wr", 1), ("Ur", 0), ("Ur", 1),
           ("Wz", 0), ("Wz", 1), ("Uz", 0), ("Uz", 1),
           ("Wh", 0), ("Wh", 1), ("Uh", 0), ("Uh", 1))
_BORDER = ("br", "bz", "bh")  # bias col = 2*gate_idx + g


def build_nc(s=S, warm_mms=None):
    """Build + compile the per-core Bass program for a shard of s rows."""
    import concourse.bass as bass
    import concourse.mybir as mybir
    import concourse.tile as tile
    from concourse import bacc

    f32 = mybir.dt.float32
    f32r = mybir.dt.float32r
    bf16 = mybir.dt.bfloat16
    AF = mybir.ActivationFunctionType
    if warm_mms is None:
        warm_mms = int(os.environ.get("GRU_WARM", "70"))

    nch = s // CH
    nc = bacc.Bacc("TRN2", target_bir_lowering=False,
                   enable_partition_id=False)
    xP = nc.dram_tensor("xP", [nch, 128, 2, CH], f32, kind="ExternalInput")
    hP = nc.dram_tensor("hP", [nch, 128, 2, CH], f32, kind="ExternalInput")
    wp_d = nc.dram_tensor("wpack", [128, 12, 256], f32, kind="ExternalInput")
    bp_d = nc.dram_tensor("bpack", [128, 6], f32, kind="ExternalInput")
    oP = nc.dram_tensor("outP", [nch, 128, 2, CH], bf16, kind="ExternalOutput")

    xPr = xP.bitcast(f32r)
    hPr = hP.bitcast(f32r)
    wp_r = wp_d.bitcast(f32r)

    with tile.TileContext(nc) as tc:
        with (
            tc.tile_pool(name="const", bufs=1) as cpool,
            tc.tile_pool(name="inp", bufs=8) as ipool,
            tc.tile_pool(name="work", bufs=3) as wpool,
            tc.tile_pool(name="psum", bufs=1, space=bass.MemorySpace.PSUM) as ppool,
        ):
            # --- PE warmup: dependency-free bf16 matmuls flip the HAM
            # clock gate to 2.4GHz while the DMA prologue runs.
            pwarm = ppool.tile([128, 64], f32, tag="pwarm")
            if warm_mms:
                warm = cpool.tile([128, 128], bf16, tag="warm")
                nc.vector.memset(warm[:], 0.0)
                for _ in range(warm_mms):
                    nc.tensor.matmul(pwarm[:], warm[:, 0:128],
                                     warm[:, 0:64], start=True, stop=True)

            wp_sb = cpool.tile([128, 12, 256], f32r, tag="wpack")
            b_sb = cpool.tile([128, 6], f32, tag="bpack")

            def load_piece(c):
                xc = ipool.tile([128, 2, CH], f32r, tag="x")
                nc.sync.dma_start(xc[:], xPr[c])
                hc = ipool.tile([128, 2, CH], f32r, tag="h")
                nc.sync.dma_start(hc[:], hPr[c])
                return xc, hc

            # prologue, phase-balanced across the two HWDGE rings so each
            # gate's weights land just before its first matmul needs them:
            #   phase1 (gates first MM): sync x0+h0 | scalar Wr,Ur + bias
            #   phase2 (z gate):        sync Wz    | scalar Uz
            #   phase3 (candidate):     sync Wh    | scalar Uh
            #   phase4+:                sync x1,h1,x2,...
            nc.scalar.dma_start(wp_sb[:, 0:4, :], wp_r[:, 0:4, :])  # Wr,Ur
            nc.scalar.dma_start(b_sb[:], bp_d[:])
            xh0 = load_piece(0)
            nc.sync.dma_start(wp_sb[:, 4:6, :], wp_r[:, 4:6, :])  # Wz
            nc.scalar.dma_start(wp_sb[:, 6:8, :], wp_r[:, 6:8, :])  # Uz
            nc.sync.dma_start(wp_sb[:, 8:10, :], wp_r[:, 8:10, :])  # Wh
            nc.scalar.dma_start(wp_sb[:, 10:12, :], wp_r[:, 10:12, :])  # Uh
            xh1 = load_piece(1)

            def wsl(j, g):
                """Stationary [128,128] slab: weight j, output half g."""
                return wp_sb[:, j, g * 128:(g + 1) * 128]

            def gate_psum(p, psl, jw, ju, rhs_w, rhs_u, g):
                """p[:, psl] = W[:,g].T @ rhs_w + U[:,g].T @ rhs_u."""
                nc.tensor.matmul(p[:, psl], wsl(jw, g), rhs_w(0),
                                 start=True, stop=False)
                nc.tensor.matmul(p[:, psl], wsl(jw + 1, g), rhs_w(1),
                                 start=False, stop=False)
                nc.tensor.matmul(p[:, psl], wsl(ju, g), rhs_u(0),
                                 start=False, stop=False)
                nc.tensor.matmul(p[:, psl], wsl(ju + 1, g), rhs_u(1),
                                 start=False, stop=True)

            def palloc(tag):
                return ppool.tile([128, CH], f32, tag=tag, name=tag)

            def mk(piece, lo, w):
                xc, hc = piece
                csl = slice(lo, lo + w)
                return (lambda k: xc[:, k, csl],
                        lambda k: hc[:, k, csl],
                        lambda k: hc[:, k, csl].bitcast(f32))

            def r_gate(xk, hk, hf, psl, g):
                pr = palloc(f"pr{g}")
                gate_psum(pr, psl, 0, 2, xk, hk, g)
                rt = wpool.tile([128, CH], f32, tag=f"r{g}", name="rt")
                nc.scalar.activation(rt[:, psl], pr[:, psl], AF.Sigmoid,
                                     bias=b_sb[:, g:g + 1])
                t = wpool.tile([128, CH], f32r, tag=f"rh{g}", name="rhr")
                nc.vector.tensor_mul(t[:, psl], rt[:, psl], hf(g))
                return t

            def z_gate(xk, hk, psl, g):
                pz = palloc(f"pz{g}")
                gate_psum(pz, psl, 4, 6, xk, hk, g)
                t = wpool.tile([128, CH], f32, tag=f"z{g}", name="zt")
                nc.scalar.activation(t[:, psl], pz[:, psl], AF.Sigmoid,
                                     bias=b_sb[:, 2 + g:3 + g])
                return t

            def h_gate(xk, rh, hf, zt, o, psl, g):
                """candidate + bf16 blend into o[:, g, psl]"""
                ph = palloc(f"ph{g}")
                gate_psum(ph, psl, 8, 10, xk, lambda k: rh[k][:, psl], g)
                hh = wpool.tile([128, CH], f32, tag=f"hh{g}", name="hh")
                nc.scalar.activation(hh[:, psl], ph[:, psl], AF.Tanh,
                                     bias=b_sb[:, 4 + g:5 + g])
                dt_ = wpool.tile([128, CH], f32, tag=f"d{g}", name="dt")
                nc.vector.tensor_sub(dt_[:, psl], hf(g), hh[:, psl])
                mt = wpool.tile([128, CH], f32, tag=f"m{g}", name="mt")
                nc.vector.tensor_mul(mt[:, psl], zt[g][:, psl], dt_[:, psl])
                nc.vector.tensor_add(o[:, g, psl], hh[:, psl], mt[:, psl])

            def compute(piece, c, lo, w, store_eng):
                """One compute chunk: batch cols [lo, lo+w) of piece c."""
                xk, hk, hf = mk(piece, lo, w)
                psl = slice(0, w)
                rh = [r_gate(xk, hk, hf, psl, g) for g in range(2)]
                zt = [z_gate(xk, hk, psl, g) for g in range(2)]
                o = wpool.tile([128, 2, CH], bf16, tag="o", name="ot")
                for g in range(2):
                    h_gate(xk, rh, hf, zt, o, psl, g)
                store_eng.dma_start(oP[c, :, :, lo:lo + w], o[:, :, psl])

            def compute_tail(items):
                """Tail items (piece, c, lo, w, store_eng) phase-interleaved:
                all r gates first, then all z, then all candidates, so the
                rh muls reach the DVE queue before any combine work and the
                PE never drains waiting on ACT/DVE products (fp32r keeps
                full PE rate down to moving-dim 256)."""
                rh, zt = {}, {}
                for i, (piece, c, lo, w, eng) in enumerate(items):
                    xk, hk, hf = mk(piece, lo, w)
                    psl = slice(0, w)
                    for g in range(2):
                        rh[i, g] = r_gate(xk, hk, hf, psl, g)
                for i, (piece, c, lo, w, eng) in enumerate(items):
                    xk, hk, hf = mk(piece, lo, w)
                    psl = slice(0, w)
                    zt[i] = [z_gate(xk, hk, psl, g) for g in range(2)]
                for i, (piece, c, lo, w, eng) in enumerate(items):
                    xk, hk, hf = mk(piece, lo, w)
                    psl = slice(0, w)
                    o = wpool.tile([128, 2, CH], bf16, tag="o", name="ot")
                    for g in range(2):
                        h_gate(xk, [rh[i, 0], rh[i, 1]], hf, zt[i], o, psl, g)
                    eng.dma_start(oP[c, :, :, lo:lo + w], o[:, :, psl])

            tail = []
            for c in range(nch):
                if c == 0:
                    piece = xh0
                elif c == 1:
                    piece = xh1
                else:
                    piece = load_piece(c)

                if c < nch - 2:
                    compute(piece, c, 0, CH, nc.gpsimd)
                elif c == nch - 2:
                    tail.append((piece, c, 0, CH // 2, nc.gpsimd))
                    tail.append((piece, c, CH // 2, CH // 2, nc.gpsimd))
                else:
                    tail.append((piece, c, 0, CH // 2, nc.sync))
                    tail.append((piece, c, CH // 2, CH // 2, nc.scalar))
            compute_tail(tail)

    nc.compile()
    return nc


_NC_CACHE = {}


def _get_nc():
    key = (S, os.environ.get("GRU_WARM", "70"))
    if key not in _NC_CACHE:
        _NC_CACHE[key] = build_nc(S)
    return _NC_CACHE[key]


def _make_in_maps(inputs):
    f32 = np.float32
    x = np.asarray(inputs["x"], f32)
    h = np.asarray(inputs["h_t_1"], f32)
    wpack = np.empty((128, 12, 256), f32)
    for j, (name, k) in enumerate(_WORDER):
        wpack[:, j, :] = np.asarray(inputs[name], f32)[k * 128:(k + 1) * 128, :]
    bpack = np.empty((128, 6), f32)
    for i, name in enumerate(_BORDER):
        b = np.asarray(inputs[name], f32)
        for g in range(2):
            bpack[:, 2 * i + g] = b[g * 128:(g + 1) * 128]
    consts = {"wpack": np.ascontiguousarray(wpack),
              "bpack": np.ascontiguousarray(bpack)}

    def pieces(a_shard):
        # [s, 256] -> [nch, 128, 2, CH]:  P[c, p, k, m] = a[c*CH+m, k*128+p]
        v = a_shard.T.reshape(2, 128, NCH, CH)  # [k, p, c, m]
        return np.ascontiguousarray(v.transpose(2, 1, 0, 3))

    in_maps = []
    for c in range(N_CORES):
        sl = slice(c * S, (c + 1) * S)
        m = {"xP": pieces(x[sl]), "hP": pieces(h[sl])}
        m.update(consts)
        in_maps.append(m)
    return in_maps


def run(inputs, trace=False):
    """Run on hardware; returns (h_t ndarray, BassKernelResults)."""
    from concourse.bass_utils import run_bass_kernel_spmd

    nc = _get_nc()
    in_maps = _make_in_maps(inputs)
    res = run_bass_kernel_spmd(nc, in_maps, list(range(N_CORES)), trace=trace)
    out = np.empty((B, D), np.float32)
    for c in range(N_CORES):
        oP = np.asarray(res.results[c]["outP"]).astype(np.float32)
        # out[c*S + cc*CH + m, g*128 + p] = oP[cc, p, g, m]
        out[c * S:(c + 1) * S] = oP.transpose(0, 3, 2, 1).reshape(S, D)
    return out, res


def kernel(**inputs):
    out, _ = run(inputs, trace=False)
    return (out, out)
